# revision 1
# baseline (speedup 1.0000x reference)
"""Trainium2 Bass kernel for a batch-first unrolled LSTM (nn_BaseRNN).

Reference computation (per batch element b, zero initial state):
    xg[t]   = x[t] @ Wx + b                      # [T, 4H], gate order (i, f, g, o)
    gates_t = xg[t] + h_{t-1} @ Wh
    i, f, g, o = split(gates_t)
    c_t = sigmoid(f) * c_{t-1} + sigmoid(i) * tanh(g)
    h_t = sigmoid(o) * tanh(c_t)
Returns (hs, cs), each [B, T, H].

Shapes: B=64, T=2048, D=H=128, 4H=512.  8 NeuronCores, data-parallel over
batch (B_LOC = 8 per core).

Design (per core):
  * Gate-major on-chip layout: tiles are [128 (gate/hidden dim), batch]
    so the tiny per-step elementwise work uses all 128 lanes.
  * The batch-8 is split into 2 independent "chains" of 4 so the two serial
    dependency chains interleave on the engines (hides per-step latency).
  * Per chain, PSUM holds one bank per gate for a 128-step chunk:
    xg for the whole chunk is matmul'd into PSUM (start=True), then each
    scan step's  h_{t-1} @ Wh_g  accumulates into its [128, 4] column slice
    (start=False).  ScalarE applies sigmoid straight out of PSUM.
  * All four gates go through ONE sigmoid instruction per step: the g-gate
    columns of Wx/Wh/b are pre-scaled by 2 on the host, and
    tanh(g) = 2*sigmoid(2g) - 1 is rebuilt with one fused DVE op.
  * Recurrent weights Wh and the h state are fp16 (fast PE weight loads);
    everything else accumulates in fp32.
  * Outputs are written HBM-transposed ([H, T, B]) for full-bandwidth DMA;
    the host re-layouts to [B, T, H].
"""

import numpy as np
from contextlib import ExitStack

import concourse.bacc as bacc
import concourse.bass as bass
import concourse.mybir as mybir
import concourse.tile as tile
from concourse import bass_utils

F32 = mybir.dt.float32
F16 = mybir.dt.float16
AF = mybir.ActivationFunctionType
OP = mybir.AluOpType

B_TOT, T_FULL, D, H = 64, 2048, 128, 128
G4 = 4 * H                      # 512
NCORES = 8
B_LOC = B_TOT // NCORES         # 8
NCHAIN = 2
B_CH = B_LOC // NCHAIN          # 4
TC = 512 // B_CH                # 128 steps/chunk: one PSUM bank per gate


def build_lstm_nc(T: int = T_FULL, with_bias: bool = False,
                  parity: bool = True) -> bacc.Bacc:
    nchunk = T // TC
    assert nchunk * TC == T

    nc = bacc.Bacc("TRN2", target_bir_lowering=False, debug=False,
                   num_devices=NCORES)

    x_d = nc.dram_tensor("x", [B_LOC, T, D], F32, kind="ExternalInput").ap()
    wx_d = nc.dram_tensor("wx", [D, G4], F32, kind="ExternalInput").ap()
    wh_d = nc.dram_tensor("wh", [H, G4], F16, kind="ExternalInput").ap()
    id_d = nc.dram_tensor("ident", [128, 128], F32, kind="ExternalInput").ap()
    if with_bias:
        b_d = nc.dram_tensor("bvec", [1, G4], F32, kind="ExternalInput").ap()
    hs_d = nc.dram_tensor("hsT", [H, T, B_LOC], F16, kind="ExternalOutput").ap()
    cs_d = nc.dram_tensor("csT", [H, T, B_LOC], F32, kind="ExternalOutput").ap()

    # Persistent SBUF
    wx_sb = nc.alloc_sbuf_tensor("wx_sb", [128, G4], F32).ap()
    wh_sb = nc.alloc_sbuf_tensor("wh_sb", [128, G4], F16).ap()
    id_sb = nc.alloc_sbuf_tensor("id_sb", [128, 128], F32).ap()
    if with_bias:
        b_sb = nc.alloc_sbuf_tensor("b_sb", [1, G4], F32).ap()
        ones_sb = nc.alloc_sbuf_tensor("ones_sb", [1, G4], F32).ap()
    npar = 2 if parity else 1
    def _temps(base, cols):
        if parity:
            return [[nc.alloc_sbuf_tensor(f"{base}{c}_{p}", [128, cols],
                                          F32).ap() for p in range(npar)]
                    for c in range(NCHAIN)]
        return [[nc.alloc_sbuf_tensor(f"{base}{c}", [128, cols], F32).ap()]
                for c in range(NCHAIN)]
    sig = _temps("sig", 4 * B_CH)
    gt = _temps("gt", B_CH)
    aa = _temps("aa", B_CH)
    mm = _temps("mm", B_CH)
    th = _temps("th", B_CH)
    h0 = [nc.alloc_sbuf_tensor(f"h0{c}", [128, B_CH], F16).ap()
          for c in range(NCHAIN)]
    c0 = [nc.alloc_sbuf_tensor(f"c0{c}", [128, B_CH], F32).ap()
          for c in range(NCHAIN)]

    # 4 PSUM banks per chain (gate g of the current chunk lives in bank g)
    psum = [nc.alloc_psum_tensor(f"ps{c}", [128, 4 * 512], F32).ap()
            for c in range(NCHAIN)]
    psg = [p.rearrange("p (g q) -> p g q", g=4) for p in psum]

    with tile.TileContext(nc) as tc_ctx, ExitStack() as ctx:
        xs_pool = [ctx.enter_context(tc_ctx.tile_pool(name=f"xs{c}", bufs=3))
                   for c in range(NCHAIN)]
        xt_pool = [ctx.enter_context(tc_ctx.tile_pool(name=f"xt{c}", bufs=2))
                   for c in range(NCHAIN)]
        hh_pool = [ctx.enter_context(tc_ctx.tile_pool(name=f"hh{c}", bufs=3))
                   for c in range(NCHAIN)]
        ch_pool = [ctx.enter_context(tc_ctx.tile_pool(name=f"ch{c}", bufs=3))
                   for c in range(NCHAIN)]

        # ---- prologue: weights, identity, state init
        nc.sync.dma_start(wx_sb, wx_d)
        nc.sync.dma_start(wh_sb, wh_d)
        nc.sync.dma_start(id_sb, id_d)
        if with_bias:
            nc.sync.dma_start(b_sb, b_d)
            nc.gpsimd.memset(ones_sb, 1.0)
        for c in range(NCHAIN):
            nc.gpsimd.memset(h0[c], 0.0)
            nc.gpsimd.memset(c0[c], 0.0)

        def load_x(c, k, xs_tile):
            for j in range(B_CH):
                nc.sync.dma_start(
                    xs_tile[:, j * 128:(j + 1) * 128],
                    x_d[c * B_CH + j, k * TC:(k + 1) * TC, :])

        hprev = [h0[c] for c in range(NCHAIN)]
        cprev = [c0[c] for c in range(NCHAIN)]

        xs_cur = []
        for c in range(NCHAIN):
            t_ = xs_pool[c].tile([128, B_CH * 128], F32, tag="xs", name=f"xs_t{c}")
            load_x(c, 0, t_)
            xs_cur.append(t_)

        for k in range(nchunk):
            hh, chist = [], []
            for c in range(NCHAIN):
                # --- chunk prologue ("bubble"): transpose x, xg matmuls
                xs_t = xs_cur[c]
                for j in range(B_CH):
                    nc.tensor.matmul(
                        psum[c][:, j * 128:(j + 1) * 128],
                        xs_t[:, j * 128:(j + 1) * 128],
                        id_sb,
                        is_transpose=True,
                        start=(j == 0), stop=(j == B_CH - 1))
                xt_t = xt_pool[c].tile([128, 512], F32, tag="xt", name=f"xt_t{c}")
                nc.vector.tensor_copy(xt_t[:, :], psum[c][:, 0:512])
                # xt columns are b-major ([d, b*128+t]); the PSUM gate banks
                # are t-major ([t*B_CH+b]) — stream the rhs t-outer/b-inner.
                xt_ap = xt_t[:, :].rearrange("p (b t) -> p t b", b=B_CH)
                for g in range(4):
                    nc.tensor.matmul(
                        psum[c][:, g * 512:(g + 1) * 512],
                        wx_sb[:, g * 128:(g + 1) * 128],
                        xt_ap,
                        start=True, stop=not with_bias)
                    if with_bias:
                        nc.tensor.matmul(
                            psum[c][:, g * 512:(g + 1) * 512],
                            b_sb[:, g * 128:(g + 1) * 128],
                            ones_sb[:, 0:512],
                            start=False, stop=True)
                # prefetch next chunk's x while this chunk scans
                if k + 1 < nchunk:
                    t_ = xs_pool[c].tile([128, B_CH * 128], F32, tag="xs", name=f"xs_t{c}")
                    load_x(c, k + 1, t_)
                    xs_cur[c] = t_
                hh.append(hh_pool[c].tile([128, TC * B_CH], F16, tag="hh", name=f"hh_t{c}"))
                chist.append(ch_pool[c].tile([128, TC * B_CH], F32, tag="ch", name=f"ch_t{c}"))

            # --- the scan
            for t in range(TC):
                for c in range(NCHAIN):
                    sl = slice(t * B_CH, (t + 1) * B_CH)
                    P = t % npar
                    sg, gtt, at, mt, tht = (sig[c][P], gt[c][P], aa[c][P],
                                            mm[c][P], th[c][P])
                    for g in range(4):
                        nc.tensor.matmul(
                            psum[c][:, g * 512 + t * B_CH:
                                    g * 512 + (t + 1) * B_CH],
                            wh_sb[:, g * 128:(g + 1) * 128],
                            hprev[c],
                            start=False, stop=False,
                            skip_group_check=True)
                    # sigmoid over all four gates (g pre-scaled by 2)
                    nc.scalar.activation(
                        sg.rearrange("p (g q) -> p g q", g=4),
                        psg[c][:, :, sl], AF.Sigmoid)
                    # g~ = tanh(g) = 2*sig - 1
                    nc.vector.tensor_scalar(
                        gtt, sg[:, 2 * B_CH:3 * B_CH],
                        2.0, -1.0, OP.mult, OP.add)
                    nc.vector.tensor_tensor(
                        at, sg[:, 0:B_CH], gtt, OP.mult)
                    nc.vector.tensor_tensor(
                        mt, sg[:, B_CH:2 * B_CH], cprev[c], OP.mult)
                    nc.vector.tensor_tensor(
                        chist[c][:, sl], mt, at, OP.add)
                    nc.scalar.activation(tht, chist[c][:, sl], AF.Tanh)
                    nc.vector.tensor_tensor(
                        hh[c][:, sl], sg[:, 3 * B_CH:4 * B_CH], tht,
                        OP.mult)
                    hprev[c] = hh[c][:, sl]
                    cprev[c] = chist[c][:, sl]

            # --- dump chunk outputs
            for c in range(NCHAIN):
                bsl = slice(c * B_CH, (c + 1) * B_CH)
                nc.sync.dma_start(
                    hs_d[:, k * TC:(k + 1) * TC, bsl],
                    hh[c][:, :].rearrange("p (t q) -> p t q", q=B_CH))
                nc.sync.dma_start(
                    cs_d[:, k * TC:(k + 1) * TC, bsl],
                    chist[c][:, :].rearrange("p (t q) -> p t q", q=B_CH))

    nc.compile()
    return nc


_NC_CACHE: dict = {}


def _get_nc(T: int, with_bias: bool, parity: bool = True) -> bacc.Bacc:
    key = (T, with_bias, parity)
    if key not in _NC_CACHE:
        _NC_CACHE[key] = build_lstm_nc(T, with_bias, parity)
    return _NC_CACHE[key]


def prep_inputs(x, Wx, Wh, b):
    """Host-side weight prep: pre-scale the g-gate (tanh) columns by 2."""
    wx_s = np.array(Wx, dtype=np.float32, copy=True)
    wh_s = np.array(Wh, dtype=np.float32, copy=True)
    b_s = np.array(b, dtype=np.float32, copy=True)
    wx_s[:, 2 * H:3 * H] *= 2.0
    wh_s[:, 2 * H:3 * H] *= 2.0
    b_s[2 * H:3 * H] *= 2.0
    with_bias = bool(np.any(b_s != 0.0))
    ident = np.eye(128, dtype=np.float32)
    x = np.asarray(x, dtype=np.float32)
    in_maps = []
    for i in range(NCORES):
        m = {
            "x": np.ascontiguousarray(x[i * B_LOC:(i + 1) * B_LOC]),
            "wx": wx_s,
            "wh": wh_s.astype(np.float16),
            "ident": ident,
        }
        if with_bias:
            m["bvec"] = b_s.reshape(1, G4)
        in_maps.append(m)
    return in_maps, with_bias


def run(x, Wx, Wh, b, T=None, trace=False):
    T = T if T is not None else x.shape[1]
    in_maps, with_bias = prep_inputs(x, Wx, Wh, b)
    nc = _get_nc(T, with_bias)
    res = bass_utils.run_bass_kernel_spmd(
        nc, in_maps, list(range(NCORES)), trace=trace)
    B = x.shape[0]
    hs = np.empty((B, T, H), dtype=np.float32)
    cs = np.empty((B, T, H), dtype=np.float32)
    for i in range(NCORES):
        hs[i * B_LOC:(i + 1) * B_LOC] = (
            res.results[i]["hsT"].astype(np.float32).transpose(2, 1, 0))
        cs[i * B_LOC:(i + 1) * B_LOC] = (
            res.results[i]["csT"].transpose(2, 1, 0))
    return (hs, cs), res


def kernel(x, Wx, Wh, b):
    (hs, cs), _ = run(x, Wx, Wh, b)
    return hs, cs



# revision 5
# speedup vs baseline: 4.4754x; 4.4754x over previous
"""Trainium2 Bass kernel for a batch-first unrolled LSTM (nn_BaseRNN).

Reference computation (per batch element b, zero initial state):
    xg[t]   = x[t] @ Wx + b                      # [T, 4H], gate order (i, f, g, o)
    gates_t = xg[t] + h_{t-1} @ Wh
    i, f, g, o = split(gates_t)
    c_t = sigmoid(f) * c_{t-1} + sigmoid(i) * tanh(g)
    h_t = sigmoid(o) * tanh(c_t)
Returns (hs, cs), each [B, T, H].

Shapes: B=64, T=2048, D=H=128, 4H=512.  8 NeuronCores, data-parallel over
batch (B_LOC = 8 per core).

Design (per core) — v2, single-chain latency-optimized:
  * ONE dependency chain covering all 8 local batch elements ([128, 8]
    tiles, hidden dim on partitions).  The wall-clock of a serial scan is
    T * (per-step critical-path latency); extra chains cannot reduce it but
    do add head-of-line blocking in the in-order engine queues, so we use
    exactly one.
  * PSUM double buffering: bank set A (banks 0-3) holds the current chunk's
    per-gate accumulators, set B (banks 4-7) is pre-filled with xg for the
    next chunk while the current chunk scans.  TC = 64 steps/chunk
    (64 steps * 8 batch = 512 fp32 = one bank per gate).
  * x is transposed on the HOST to [D, T, B] so xt tiles DMA directly as
    [128, TC*8] with 2KB contiguous lines — no PE transpose, no PSUM->SBUF
    copy on device.
  * Per step: 4 matmuls (h_{t-1} @ Wh_g accumulated onto xg in PSUM,
    fp16 weights), ONE sigmoid over all 4 gates (g columns pre-scaled by 2
    on host; tanh(g) = 2*sigmoid(2g) - 1 rebuilt with one fused DVE op),
    fused DVE elementwise chain, one tanh, one output multiply.
  * xg matmuls for chunk k+1 are split into 64-column pieces interleaved
    one-per-step into the PE queue so they fill PE idle time without
    blocking the scan's step matmuls.
  * Outputs are written HBM-transposed ([H, T, B]) for full-bandwidth DMA;
    the host re-layouts to [B, T, H].
"""

import numpy as np
from contextlib import ExitStack

import concourse.bacc as bacc
import concourse.bass as bass
import concourse.mybir as mybir
import concourse.tile as tile
from concourse import bass_utils

F32 = mybir.dt.float32
F16 = mybir.dt.float16
F32R = mybir.dt.float32r
AF = mybir.ActivationFunctionType
OP = mybir.AluOpType

B_TOT, T_FULL, D, H = 64, 2048, 128, 128
G4 = 4 * H                      # 512
NCORES = 8
B_LOC = B_TOT // NCORES         # 8
TC = 512 // B_LOC               # 64 steps/chunk: one PSUM bank per gate
NPAR = 2
XG_PIECES = 1                   # sub-matmuls per gate (start=True resets whole bank)


def build_lstm_nc(T: int = T_FULL, with_bias: bool = False) -> bacc.Bacc:
    nchunk = T // TC
    assert nchunk * TC == T and nchunk >= 2

    nc = bacc.Bacc("TRN2", target_bir_lowering=False, debug=False,
                   num_devices=NCORES)

    xt_d = nc.dram_tensor("xT", [D, T, B_LOC], F32R, kind="ExternalInput").ap()
    wx_d = nc.dram_tensor("wx", [D, G4], F32R, kind="ExternalInput").ap()
    wh_d = nc.dram_tensor("wh", [H, G4], F16, kind="ExternalInput").ap()
    if with_bias:
        b_d = nc.dram_tensor("bvec", [1, G4], F32, kind="ExternalInput").ap()
    hs_d = nc.dram_tensor("hsT", [H, T, B_LOC], F16, kind="ExternalOutput").ap()
    cs_d = nc.dram_tensor("csT", [H, T, B_LOC], F32, kind="ExternalOutput").ap()

    # Persistent SBUF
    wx_sb = nc.alloc_sbuf_tensor("wx_sb", [128, G4], F32R).ap()
    wh_sb = nc.alloc_sbuf_tensor("wh_sb", [128, G4], F16).ap()
    if with_bias:
        b_sb = nc.alloc_sbuf_tensor("b_sb", [1, G4], F32).ap()
        ones_sb = nc.alloc_sbuf_tensor("ones_sb", [1, G4], F32).ap()
    sg = [nc.alloc_sbuf_tensor(f"sg{p}", [128, 4 * B_LOC], F32).ap()
          for p in range(NPAR)]
    sgv = [s.rearrange("p (g q) -> p g q", g=4) for s in sg]
    gt = [nc.alloc_sbuf_tensor(f"gt{p}", [128, B_LOC], F32).ap()
          for p in range(NPAR)]
    at = [nc.alloc_sbuf_tensor(f"at{p}", [128, B_LOC], F32).ap()
          for p in range(NPAR)]
    mt = [nc.alloc_sbuf_tensor(f"mt{p}", [128, B_LOC], F32).ap()
          for p in range(NPAR)]
    th = [nc.alloc_sbuf_tensor(f"th{p}", [128, B_LOC], F32).ap()
          for p in range(NPAR)]
    h0 = nc.alloc_sbuf_tensor("h0", [128, B_LOC], F16).ap()
    c0 = nc.alloc_sbuf_tensor("c0", [128, B_LOC], F32).ap()

    # All 8 PSUM banks as one tensor: set s in cols [s*2048, (s+1)*2048),
    # gate g at +g*512, step t at +t*8.
    ps = nc.alloc_psum_tensor("ps", [128, 8 * 512], F32).ap()
    psv = ps.rearrange("p (s g q) -> p s g q", s=2, g=4)

    with tile.TileContext(nc) as tc_ctx, ExitStack() as ctx:
        xt_pool = ctx.enter_context(tc_ctx.tile_pool(name="xt", bufs=3))
        hh_pool = ctx.enter_context(tc_ctx.tile_pool(name="hh", bufs=3))
        ch_pool = ctx.enter_context(tc_ctx.tile_pool(name="ch", bufs=3))

        # ---- prologue: weights, state init, chunk-0 xg
        nc.sync.dma_start(wx_sb, wx_d)
        nc.sync.dma_start(wh_sb, wh_d)
        if with_bias:
            nc.sync.dma_start(b_sb, b_d)
            nc.gpsimd.memset(ones_sb, 1.0)
        nc.gpsimd.memset(h0, 0.0)
        nc.gpsimd.memset(c0, 0.0)

        def load_xt(k, xt_tile):
            nc.sync.dma_start(
                xt_tile[:, :].rearrange("p (t q) -> p t q", q=B_LOC),
                xt_d[:, k * TC:(k + 1) * TC, :])

        def xg_mm(sset, g, c0_, c1_, xt_tile):
            nc.tensor.matmul(
                ps[:, sset * 2048 + g * 512 + c0_:sset * 2048 + g * 512 + c1_],
                wx_sb[:, g * 128:(g + 1) * 128],
                xt_tile[:, c0_:c1_],
                start=True, stop=not with_bias)
            if with_bias:
                nc.tensor.matmul(
                    ps[:, sset * 2048 + g * 512 + c0_:
                       sset * 2048 + g * 512 + c1_],
                    b_sb[:, g * 128:(g + 1) * 128],
                    ones_sb[:, c0_:c1_],
                    start=False, stop=True)

        xt_cur = xt_pool.tile([128, TC * B_LOC], F32R, tag="xt", name="xt_t")
        load_xt(0, xt_cur)
        for g in range(4):
            xg_mm(0, g, 0, 512, xt_cur)
        xt_next = xt_pool.tile([128, TC * B_LOC], F32R, tag="xt", name="xt_t")
        load_xt(1, xt_next)

        hh_prev = None
        ch_prev = None
        for k in range(nchunk):
            s = k % 2
            base = s * 2048
            hh = hh_pool.tile([128, TC * B_LOC], F16, tag="hh", name="hh_t")
            ch = ch_pool.tile([128, TC * B_LOC], F32, tag="ch", name="ch_t")

            # xg pieces for chunk k+1 (into the other bank set), one per
            # step starting at t=2: 4 gates x 8 pieces of 64 cols.
            pieces = []
            if k + 1 < nchunk:
                for g in range(4):
                    for p_ in range(XG_PIECES):
                        w = 512 // XG_PIECES
                        pieces.append((1 - s, g, p_ * w, (p_ + 1) * w))

            for t in range(TC):
                if k == 0 and t == 0:
                    hprev, cprev = h0, c0
                elif t == 0:
                    hprev = hh_prev[:, (TC - 1) * B_LOC:TC * B_LOC]
                    cprev = ch_prev[:, (TC - 1) * B_LOC:TC * B_LOC]
                else:
                    hprev = hh[:, (t - 1) * B_LOC:t * B_LOC]
                    cprev = ch[:, (t - 1) * B_LOC:t * B_LOC]
                P = t % NPAR
                sl = slice(t * B_LOC, (t + 1) * B_LOC)

                for g in range(4):
                    nc.tensor.matmul(
                        ps[:, base + g * 512 + t * B_LOC:
                           base + g * 512 + (t + 1) * B_LOC],
                        wh_sb[:, g * 128:(g + 1) * 128],
                        hprev,
                        start=False, stop=False,
                        skip_group_check=True)
                # interleaved PE work for the next chunk
                if pieces and t >= 2 and (t - 2) % 16 == 0 \
                        and (t - 2) // 16 < len(pieces):
                    ss, g_, a_, b_ = pieces[(t - 2) // 16]
                    xg_mm(ss, g_, a_, b_, xt_next)
                if t == 1 and k + 2 < nchunk:
                    t_ = xt_pool.tile([128, TC * B_LOC], F32R, tag="xt",
                                      name="xt_t")
                    load_xt(k + 2, t_)
                    xt_after = t_

                # sigmoid over all four gates (g pre-scaled by 2)
                nc.scalar.activation(sgv[P], psv[:, s, :, sl], AF.Sigmoid)
                # g~ = tanh(g) = 2*sig - 1
                nc.vector.tensor_scalar(
                    gt[P], sgv[P][:, 2, :], 2.0, -1.0, OP.mult, OP.add)
                nc.vector.tensor_tensor(at[P], sgv[P][:, 0, :], gt[P], OP.mult)
                nc.vector.tensor_tensor(mt[P], sgv[P][:, 1, :], cprev, OP.mult)
                nc.vector.tensor_tensor(ch[:, sl], at[P], mt[P], OP.add)
                nc.scalar.activation(th[P], ch[:, sl], AF.Tanh)
                nc.vector.tensor_tensor(hh[:, sl], sgv[P][:, 3, :], th[P],
                                        OP.mult)

            # --- dump chunk outputs, rotate prefetch tiles
            nc.sync.dma_start(
                hs_d[:, k * TC:(k + 1) * TC, :],
                hh[:, :].rearrange("p (t q) -> p t q", q=B_LOC))
            nc.sync.dma_start(
                cs_d[:, k * TC:(k + 1) * TC, :],
                ch[:, :].rearrange("p (t q) -> p t q", q=B_LOC))
            hh_prev, ch_prev = hh, ch
            if k + 1 < nchunk:
                xt_cur = xt_next
                if k + 2 < nchunk:
                    xt_next = xt_after

    nc.compile()
    return nc


_NC_CACHE: dict = {}


def _get_nc(T: int, with_bias: bool) -> bacc.Bacc:
    key = (T, with_bias)
    if key not in _NC_CACHE:
        _NC_CACHE[key] = build_lstm_nc(T, with_bias)
    return _NC_CACHE[key]


def prep_inputs(x, Wx, Wh, b):
    """Host-side prep: pre-scale the g-gate (tanh) columns by 2, transpose
    x to [D, T, B] per core."""
    wx_s = np.array(Wx, dtype=np.float32, copy=True)
    wh_s = np.array(Wh, dtype=np.float32, copy=True)
    b_s = np.array(b, dtype=np.float32, copy=True)
    wx_s[:, 2 * H:3 * H] *= 2.0
    wh_s[:, 2 * H:3 * H] *= 2.0
    b_s[2 * H:3 * H] *= 2.0
    with_bias = bool(np.any(b_s != 0.0))
    x = np.asarray(x, dtype=np.float32)
    wh_f16 = wh_s.astype(np.float16)
    in_maps = []
    for i in range(NCORES):
        m = {
            "xT": np.ascontiguousarray(
                x[i * B_LOC:(i + 1) * B_LOC].transpose(2, 1, 0)),
            "wx": wx_s,
            "wh": wh_f16,
        }
        if with_bias:
            m["bvec"] = b_s.reshape(1, G4)
        in_maps.append(m)
    return in_maps, with_bias


def run(x, Wx, Wh, b, T=None, trace=False):
    T = T if T is not None else x.shape[1]
    in_maps, with_bias = prep_inputs(x, Wx, Wh, b)
    nc = _get_nc(T, with_bias)
    res = bass_utils.run_bass_kernel_spmd(
        nc, in_maps, list(range(NCORES)), trace=trace)
    B = x.shape[0]
    hs = np.empty((B, T, H), dtype=np.float32)
    cs = np.empty((B, T, H), dtype=np.float32)
    for i in range(NCORES):
        hs[i * B_LOC:(i + 1) * B_LOC] = (
            res.results[i]["hsT"].astype(np.float32).transpose(2, 1, 0))
        cs[i * B_LOC:(i + 1) * B_LOC] = (
            res.results[i]["csT"].transpose(2, 1, 0))
    return (hs, cs), res


def kernel(x, Wx, Wh, b):
    (hs, cs), _ = run(x, Wx, Wh, b)
    return hs, cs
